# revision 42
# baseline (speedup 1.0000x reference)
"""Trainium2 Bass kernel for nn_MediumRangeEdge (retrieval_knn).

For each batch graph: L2-normalize node features, pairwise distance
dist = 2 - 2*x@x.T + relative_pos + INF*mask, top-10 smallest per node,
emit edge list [dst, src, 0].

Distribution: data-parallel over batch. 32 graphs -> 8 NeuronCores, 4
graphs per core. No cross-device communication.

Score decomposition: with unit-norm features, ranking smallest dist per
row == ranking largest s[i,j] = 4096*x^_i.x^_j + 8*pe_i.pe_j (uniform
constants drop out). pe_i.pe_j = S[c_i,c_j] + S[r_i,r_j] where S is a
28x28 PSD Toeplitz Gram of the sincos vectors, so the positional part
factors EXACTLY into 56 extra contraction dims phi (batch-independent,
shipped fp8 in DoubleRow ktile layout).

The 28 row-units (4 graphs x 7 uniform 112-row units) run as 12 PAIRS
with two solo units at each end (finer fill/drain granularity); each
pair gets one fused elementwise op per stage, halving per-op fixed
costs. Pair psum is one [128,2048] f32 tile (4 banks, double-buffered;
member m at col m*1024 so both members keep bank-aligned col blocks).

Device pipeline per pair:
  PE    psum = 4096*x@x.T via fp8e4 DoubleRow matmuls (features hold
        64*x/||x||; 2 packed-256 contractions per col-block) + one
        DoubleRow phi matmul (positional bias) + a banded DoubleRow
        mask matmul (96*I x -224 band = -21504) pushing diag+8-neighbor
        entries below every real score. No elementwise bias pass.
  ACT   int16(psum) -> HIGH halves of a pre-tagged int32 key tile
        (stride-2 write); the f32->i16 convert is the score quantizer.
        Low halves hold the permanent tie-break tag u = 64*(1023-col),
        DMA'd once from host as exact int32. The 64 spacing makes tags
        recoverable after engine float datapaths round int32 keys to
        f32 (ulp <= 32 below 2^29; measured on-device - the HW iota
        has the same rounding, hence host-built templates).
        key = i16(score)<<16 | u: int32 order = quantized-score order
        with ties toward lower column (matching jax.lax.top_k).
  DVE   fold h = max(key[:392], key[392:]) (mod-392 classes, 2 members
        - GPSIMD has no max opcode, so folding cannot use Pool), then
        top-8 of each 196-wide h half via max8; host merges 16, keeps
        10. A true top-10 entry is lost only if two collide mod 392 or
        rank >8 within a half (measured together ~5.9e-3 rel err vs
        the 2e-2 budget, fp8 noise included).
  Host  decodes col = 1023 - round((key & 0xFFFF)/64) from the DMA'd
        key lists (rounding-robust against the f32 key rounding).
"""
import sys

if "/opt/trn_rl_repo" not in sys.path:
    sys.path.insert(0, "/opt/trn_rl_repo")

import numpy as np
import ml_dtypes

BATCH = 32
N = 784  # 28*28 nodes
D = 512
K = 10
RES = 28
NCORES = 8
BPC = BATCH // NCORES

P = 128
U = 7  # uniform row units
UR = 112  # rows per unit (7*112 = 784)
NS = BPC * U  # 28 units per core
CBLKS = [(0, 256), (256, 256), (512, 272)]  # psum col blocks (bank-safe)
MW = 170  # mask band width (58 + 112)
FSCALE = 64.0  # feature scale; score products are 4096*s
ACOLS = 784  # cols per member quantized on ACT (DVE split recreates a fold-queue hazard under the static tile scheduler)
NKEY = 2

F8NP = ml_dtypes.float8_e4m3
BF16NP = ml_dtypes.bfloat16

_CACHE = {}


def _mask_np():
    idx = np.arange(N)
    r, c = idx // RES, idx % RES
    mask = np.zeros((N, N), np.float32)
    for dr, dc in [(0, -1), (0, 1), (-1, 0), (1, 0), (-1, -1), (-1, 1), (1, -1), (1, 1)]:
        rr, cc = r + dr, c + dc
        valid = (rr >= 0) & (rr < RES) & (cc >= 0) & (cc < RES)
        mask[idx[valid], (rr * RES + cc)[valid]] = 1.0
    mask[idx, idx] = 1.0
    return mask


def _mask_j0(u):
    return min(max(u * UR - 29, 0), N - MW)


def _phi_np():
    """[N, 56] float64: phi_i . phi_j == 8 * pe_i . pe_j exactly."""
    omega = np.arange(128, dtype=np.float64) / 128.0
    omega = 1.0 / 10000.0**omega
    pos = np.arange(RES, dtype=np.float64)
    sv = np.concatenate(
        [np.sin(np.outer(pos, omega)), np.cos(np.outer(pos, omega))], axis=1
    )
    S = sv @ sv.T  # [28, 28] PSD
    w, V = np.linalg.eigh(S)
    G = V * np.sqrt(np.clip(w, 0.0, None))[None, :]
    idx = np.arange(N)
    r, c = idx // RES, idx % RES
    phi = np.zeros((N, 56))
    phi[:, :28] = np.sqrt(8.0) * G[c]  # grid[0] ("emb_h") is the col coord
    phi[:, 28:] = np.sqrt(8.0) * G[r]
    return phi


def build_bass():
    import concourse.bacc as bacc
    import concourse.mybir as mybir
    from concourse.tile import TileContext
    from contextlib import ExitStack

    f32 = mybir.dt.float32
    i32 = mybir.dt.int32
    i16 = mybir.dt.int16
    f8 = mybir.dt.float8e4
    bf16 = mybir.dt.bfloat16
    AF = mybir.ActivationFunctionType
    AL = mybir.AluOpType
    DR = mybir.MatmulPerfMode.DoubleRow

    nc = bacc.Bacc("TRN2", target_bir_lowering=False, debug=False, num_devices=NCORES)
    # fp8 features, ktile layout: x8[b, p, i*784 + n] = xh8[b, n, i*128 + p]
    x8d = nc.declare_dram_parameter("x8", [BPC, P, 4 * N], f8, isOutput=False)
    # [56-part consts, DoubleRow ktile layouts] 96*I(112) ++ mask bands ++
    # fp8 positional factors (rows 0:28): one DMA covers all three
    CPW = 2 * UR + 2 * U * MW
    cphd = nc.declare_dram_parameter("cph", [56, CPW + 2 * N], f8, isOutput=False)
    # exact int32 tag template (HW iota rounds 10-bit values to a float grid)
    tagd = nc.declare_dram_parameter("tag", [P, 2 * N], i32, isOutput=False)
    idx_out = nc.declare_dram_parameter("idx", [BPC, UR, U * 16], i32, isOutput=True)

    # unit s = 7*b + u; groups: two solos (fill: each needs only a tag
    # half-tile), 12 pairs, then two solos so the tail drains finely
    groups = (
        [[0], [1]]
        + [[s, s + 1] for s in range(2, NS - 2, 2)]
        + [[NS - 2], [NS - 1]]
    )

    with TileContext(nc) as tc, ExitStack() as ctx:
        consts = ctx.enter_context(tc.tile_pool(name="consts", bufs=1))
        xt_pool = ctx.enter_context(tc.tile_pool(name="xt", bufs=2))
        fold_pool = ctx.enter_context(tc.tile_pool(name="fold", bufs=3))
        ps_mm = ctx.enter_context(tc.tile_pool(name="ps_mm", bufs=2, space="PSUM"))

        keyt = [consts.tile([P, 2 * N], i32, name=f"key_{i}") for i in range(NKEY)]
        cph = consts.tile([56, CPW + 2 * N], f8, name="cph")
        idx_acc = [
            consts.tile([P, U * 16], i32, name=f"idx_acc_{b}") for b in range(BPC)
        ]

        phiv = cph[:28, CPW:].rearrange("p (two n) -> p two n", two=2)
        ident = cph[:, 0 : 2 * UR].rearrange("p (two n) -> p two n", two=2)
        mskt = cph[:, 2 * UR : CPW].rearrange("p (two n) -> p two n", two=2)

        def prep(b, split):
            xt = xt_pool.tile([P, 4 * N], f8, tag="xt", name=f"x8_{b}")
            if split:
                # ktile halves: gram pair 0 can start after the first half
                nc.sync.dma_start(out=xt[:, 0 : 2 * N], in_=x8d.ap()[b, :, 0 : 2 * N])
                nc.sync.dma_start(out=xt[:, 2 * N :], in_=x8d.ap()[b, :, 2 * N :])
            else:
                nc.sync.dma_start(out=xt, in_=x8d.ap()[b])
            return xt.rearrange("p (k n) -> p k n", k=4)

        xks = {}

        def member_matmuls(ps, mi, b, u):
            xk = xks[b]
            r0 = u * UR
            co = mi * 1024
            j0 = _mask_j0(u)
            for c0, cw in CBLKS:
                nc.tensor.matmul(
                    ps[:UR, co + c0 : co + c0 + cw],
                    lhsT=xk[:, 0:2, r0 : r0 + UR],
                    rhs=xk[:, 0:2, c0 : c0 + cw],
                    start=True,
                    stop=False,
                    perf_mode=DR,
                )
                nc.tensor.matmul(
                    ps[:UR, co + c0 : co + c0 + cw],
                    lhsT=xk[:, 2:4, r0 : r0 + UR],
                    rhs=xk[:, 2:4, c0 : c0 + cw],
                    start=False,
                    stop=False,
                    perf_mode=DR,
                )
                a = max(j0, c0)
                z = min(j0 + MW, c0 + cw)
                if a < z:
                    nc.tensor.matmul(
                        ps[:UR, co + a : co + z],
                        lhsT=ident,
                        rhs=mskt[:, :, u * MW + (a - j0) : u * MW + (z - j0)],
                        start=False,
                        stop=False,
                        perf_mode=DR,
                    )
                nc.tensor.matmul(
                    ps[:UR, co + c0 : co + c0 + cw],
                    lhsT=phiv[:, :, r0 : r0 + UR],
                    rhs=phiv[:, :, c0 : c0 + cw],
                    start=False,
                    stop=True,
                    perf_mode=DR,
                )

        def emit_convert(gi, mem, ps):
            key = keyt[gi % NKEY]
            if len(mem) == 2:
                k16 = key.bitcast(i16).rearrange(
                    "p (two n half) -> p two n half", two=2, half=2
                )
                psv = ps.rearrange("p (two n) -> p two n", two=2)
                nc.scalar.activation(
                    k16[:UR, :, 0:ACOLS, 1], psv[:UR, :, 0:ACOLS], AF.Copy
                )
                if ACOLS < N:
                    with tc.high_priority():
                        nc.vector.tensor_copy(
                            k16[:UR, :, ACOLS:N, 1], psv[:UR, :, ACOLS:N]
                        )
            else:
                k16s = key.bitcast(i16).rearrange("p (n half) -> p n half", half=2)
                nc.scalar.activation(
                    k16s[:UR, 0:ACOLS, 1], ps[:UR, 0:ACOLS], AF.Copy
                )
                if ACOLS < N:
                    with tc.high_priority():
                        nc.vector.tensor_copy(
                            k16s[:UR, ACOLS:N, 1], ps[:UR, ACOLS:N]
                        )
            return key

        def emit_folds(gi, mem, key):
            # GPSIMD has no max opcode, so folding lives on DVE: one fold to
            # mod-392 classes (2 members), then top-8 of each 196-wide h half;
            # host merges 16 and keeps 10.
            h = fold_pool.tile([P, 2 * 392], i32, tag="h")
            if len(mem) == 2:
                kv = key.rearrange("p (two n) -> p two n", two=2)
                hv = h.rearrange("p (two n) -> p two n", two=2)
                nc.vector.tensor_tensor(
                    out=hv[:UR], in0=kv[:UR, :, 0:392], in1=kv[:UR, :, 392:784],
                    op=AL.max,
                )
            else:
                nc.vector.tensor_tensor(
                    out=h[:UR, 0:392], in0=key[:UR, 0:392], in1=key[:UR, 392:784],
                    op=AL.max,
                )
            for mi, s in enumerate(mem):
                b, u = divmod(s, U)
                for half in range(2):
                    nc.vector.max(
                        out=idx_acc[b][:UR, u * 16 + 8 * half : u * 16 + 8 * half + 8],
                        in_=h[:UR, mi * 392 + 196 * half : mi * 392 + 196 * half + 196],
                    )
                # stream finished key columns out; the last batch goes in
                # two pieces so the post-last-unit tail is one small DMA
                cuts = (
                    {4: (0, 80), 6: (80, 112)}
                    if b == BPC - 1
                    else {5: (0, 96), 6: (96, 112)}
                )
                if u in cuts:
                    lo, hi = cuts[u]
                    nc.sync.dma_start(
                        out=idx_out.ap()[b, :, lo:hi], in_=idx_acc[b][:UR, lo:hi]
                    )

        # ---- driver ----
        # warm the ACT function table off the critical path
        warm = consts.tile([1, 2], f32, name="warm")
        nc.vector.memset(warm, 0.0)
        nc.scalar.activation(warm, warm, AF.Copy)
        xks[0] = prep(0, split=True)
        nc.sync.dma_start(out=cph, in_=cphd.ap())
        # exact tag templates by DMA (iota's float datapath rounds 10-bit
        # tags), in criticality order: the two leading solo groups touch only
        # the first halves of key0/key1; the pairs at groups 2/3 need the rest.
        nc.sync.dma_start(out=keyt[0][:, 0:N], in_=tagd.ap()[:, 0:N])
        nc.sync.dma_start(out=keyt[1][:, 0:N], in_=tagd.ap()[:, 0:N])
        nc.sync.dma_start(out=keyt[0][:, N:], in_=tagd.ap()[:, N:])
        nc.sync.dma_start(out=keyt[1][:, N:], in_=tagd.ap()[:, N:])

        for gi, mem in enumerate(groups):
            for s in mem:
                b, u = divmod(s, U)
                if u == 2 and b + 1 < BPC:
                    xks[b + 1] = prep(b + 1, split=False)
            ps = ps_mm.tile([P, 2048], f32, tag="ps_mm")
            for mi, s in enumerate(mem):
                member_matmuls(ps, mi, *divmod(s, U))
            key = emit_convert(gi, mem, ps)
            emit_folds(gi, mem, key)

    nc.finalize()
    return nc


def _get_nc():
    if "nc" not in _CACHE:
        _CACHE["nc"] = build_bass()
    return _CACHE["nc"]


def _host_consts():
    if "consts" in _CACHE:
        return _CACHE["consts"]
    mask = _mask_np()

    # c128 (DoubleRow layouts, contract d=(p,i)=2p+i): 96*I ++ mask bands
    identT = 96.0 * np.eye(UR, dtype=np.float32).reshape(56, 2, UR)
    mskDR = np.zeros((56, 2, U * MW), np.float32)
    for u in range(U):
        j0 = _mask_j0(u)
        mskDR[:, :, u * MW : (u + 1) * MW] = -224.0 * mask[
            u * UR : (u + 1) * UR, j0 : j0 + MW
        ].reshape(56, 2, MW)
    cph = np.zeros((56, 2 * UR + 2 * U * MW + 2 * N), np.float32)
    cph[:, 0 : 2 * UR] = identT.reshape(56, 2 * UR)
    cph[:, 2 * UR : 2 * UR + 2 * U * MW] = mskDR.reshape(56, 2 * U * MW)
    cph[:28, 2 * UR + 2 * U * MW :] = _phi_np().T.reshape(28, 2 * N)
    cph = cph.astype(F8NP)

    # tags spaced 64 apart: engine float datapaths round int32 keys to f32
    # (ulp <= 32 below 2^29), so a 64 spacing keeps tags exactly recoverable
    tag = np.broadcast_to(
        np.tile(64 * (1023 - np.arange(N, dtype=np.int32)), 2)[None, :], (P, 2 * N)
    ).copy()
    _CACHE["consts"] = (cph, tag)
    return _CACHE["consts"]


def kernel(node_feature, relative_pos):
    from concourse.bass_utils import run_bass_kernel_spmd

    x = np.asarray(node_feature, dtype=np.float32)

    nrm = np.sqrt((x * x).sum(-1, dtype=np.float32), dtype=np.float32)
    nrm = np.maximum(nrm, np.float32(1e-12))
    xh8 = (x * (np.float32(FSCALE) / nrm)[..., None]).astype(F8NP)  # [B, N, D]

    # ktile layout [B, 128, 4*784]: x8[b, p, i*784+n] = xh8[b, n, i*128+p]
    x8 = np.ascontiguousarray(
        xh8.reshape(BATCH, N, 4, P).transpose(0, 3, 2, 1).reshape(BATCH, P, 4 * N)
    )

    cph, tag = _host_consts()

    nc = _get_nc()
    in_maps = [
        {
            "x8": np.ascontiguousarray(x8[i * BPC : (i + 1) * BPC]),
            "cph": cph,
            "tag": tag,
        }
        for i in range(NCORES)
    ]
    res = run_bass_kernel_spmd(nc, in_maps, list(range(NCORES)))

    topk = np.zeros((BATCH, N, K), np.int32)
    for i in range(NCORES):
        keys = res.results[i]["idx"].reshape(BPC, UR, U, 16)
        # per unit: top8(h half0) ++ top8(h half1) -> merge, keep 10
        srt = np.sort(keys, axis=-1)[:, :, :, ::-1][:, :, :, :K]
        m = 1023 - (((srt & 65535) + 32) >> 6)
        topk[i * BPC : (i + 1) * BPC] = (
            m.transpose(0, 2, 1, 3).reshape(BPC, N, K).astype(np.int32)
        )

    dst = topk + (np.arange(BATCH, dtype=np.int32) * N)[:, None, None]
    src = np.broadcast_to(
        np.arange(BATCH * N, dtype=np.int32).reshape(BATCH, N, 1), (BATCH, N, K)
    )
    relation = np.zeros_like(dst)
    return np.stack([dst, src, relation], axis=-1).reshape(-1, 3)


# revision 43
# speedup vs baseline: 1.0267x; 1.0267x over previous
"""Trainium2 Bass kernel for nn_MediumRangeEdge (retrieval_knn).

For each batch graph: L2-normalize node features, pairwise distance
dist = 2 - 2*x@x.T + relative_pos + INF*mask, top-10 smallest per node,
emit edge list [dst, src, 0].

Distribution: data-parallel over batch. 32 graphs -> 8 NeuronCores, 4
graphs per core. No cross-device communication.

Score decomposition: with unit-norm features, ranking smallest dist per
row == ranking largest s[i,j] = 4096*x^_i.x^_j + 8*pe_i.pe_j (uniform
constants drop out). pe_i.pe_j = S[c_i,c_j] + S[r_i,r_j] where S is a
28x28 PSD Toeplitz Gram of the sincos vectors, so the positional part
factors EXACTLY into 56 extra contraction dims phi (batch-independent,
shipped fp8 in DoubleRow ktile layout).

The 28 row-units (4 graphs x 7 uniform 112-row units) run as 12 PAIRS
with two solo units at each end (finer fill/drain granularity); each
pair gets one fused elementwise op per stage, halving per-op fixed
costs. Pair psum is one [128,2048] f32 tile (4 banks, double-buffered;
member m at col m*1024 so both members keep bank-aligned col blocks).

Device pipeline per pair:
  PE    psum = 4096*x@x.T via fp8e4 DoubleRow matmuls (features hold
        64*x/||x||; 2 packed-256 contractions per col-block) + one
        DoubleRow phi matmul (positional bias) + a banded DoubleRow
        mask matmul (96*I x -224 band = -21504) pushing diag+8-neighbor
        entries below every real score. No elementwise bias pass.
  ACT   int16(psum) -> HIGH halves of a pre-tagged int32 key tile
        (stride-2 write); the f32->i16 convert is the score quantizer.
        Low halves hold the permanent tie-break tag u = 64*(1023-col),
        DMA'd once from host as exact int32. The 64 spacing makes tags
        recoverable after engine float datapaths round int32 keys to
        f32 (ulp <= 32 below 2^29; measured on-device - the HW iota
        has the same rounding, hence host-built templates).
        key = i16(score)<<16 | u: int32 order = quantized-score order
        with ties toward lower column (matching jax.lax.top_k).
  DVE   fold h = max(key[:392], key[392:]) (mod-392 classes, 2 members
        - GPSIMD has no max opcode, so folding cannot use Pool), then
        top-8 of each 196-wide h half via max8; host merges 16, keeps
        10. A true top-10 entry is lost only if two collide mod 392 or
        rank >8 within a half (measured together ~5.9e-3 rel err vs
        the 2e-2 budget, fp8 noise included).
  Host  decodes col = 1023 - round((key & 0xFFFF)/64) from the DMA'd
        key lists (rounding-robust against the f32 key rounding).
"""
import sys

if "/opt/trn_rl_repo" not in sys.path:
    sys.path.insert(0, "/opt/trn_rl_repo")

import numpy as np
import ml_dtypes

BATCH = 32
N = 784  # 28*28 nodes
D = 512
K = 10
RES = 28
NCORES = 8
BPC = BATCH // NCORES

P = 128
U = 7  # uniform row units
UR = 112  # rows per unit (7*112 = 784)
NS = BPC * U  # 28 units per core
CBLKS = [(0, 256), (256, 256), (512, 272)]  # psum col blocks (bank-safe)
MW = 170  # mask band width (58 + 112)
FSCALE = 64.0  # feature scale; score products are 4096*s
ACOLS = 784  # cols per member quantized on ACT (DVE split recreates a fold-queue hazard under the static tile scheduler)
NKEY = 2

F8NP = ml_dtypes.float8_e4m3
BF16NP = ml_dtypes.bfloat16

_CACHE = {}


def _mask_np():
    idx = np.arange(N)
    r, c = idx // RES, idx % RES
    mask = np.zeros((N, N), np.float32)
    for dr, dc in [(0, -1), (0, 1), (-1, 0), (1, 0), (-1, -1), (-1, 1), (1, -1), (1, 1)]:
        rr, cc = r + dr, c + dc
        valid = (rr >= 0) & (rr < RES) & (cc >= 0) & (cc < RES)
        mask[idx[valid], (rr * RES + cc)[valid]] = 1.0
    mask[idx, idx] = 1.0
    return mask


def _mask_j0(u):
    return min(max(u * UR - 29, 0), N - MW)


def _phi_np():
    """[N, 56] float64: phi_i . phi_j == 8 * pe_i . pe_j exactly."""
    omega = np.arange(128, dtype=np.float64) / 128.0
    omega = 1.0 / 10000.0**omega
    pos = np.arange(RES, dtype=np.float64)
    sv = np.concatenate(
        [np.sin(np.outer(pos, omega)), np.cos(np.outer(pos, omega))], axis=1
    )
    S = sv @ sv.T  # [28, 28] PSD
    w, V = np.linalg.eigh(S)
    G = V * np.sqrt(np.clip(w, 0.0, None))[None, :]
    idx = np.arange(N)
    r, c = idx // RES, idx % RES
    phi = np.zeros((N, 56))
    phi[:, :28] = np.sqrt(8.0) * G[c]  # grid[0] ("emb_h") is the col coord
    phi[:, 28:] = np.sqrt(8.0) * G[r]
    return phi


def build_bass():
    import concourse.bacc as bacc
    import concourse.mybir as mybir
    from concourse.tile import TileContext
    from contextlib import ExitStack

    f32 = mybir.dt.float32
    i32 = mybir.dt.int32
    i16 = mybir.dt.int16
    f8 = mybir.dt.float8e4
    bf16 = mybir.dt.bfloat16
    AF = mybir.ActivationFunctionType
    AL = mybir.AluOpType
    DR = mybir.MatmulPerfMode.DoubleRow

    nc = bacc.Bacc("TRN2", target_bir_lowering=False, debug=False, num_devices=NCORES)
    # fp8 features, ktile layout: x8[b, p, i*784 + n] = xh8[b, n, i*128 + p]
    x8d = nc.declare_dram_parameter("x8", [BPC, P, 4 * N], f8, isOutput=False)
    # [56-part consts, DoubleRow ktile layouts] 96*I(112) ++ mask bands ++
    # fp8 positional factors (rows 0:28): one DMA covers all three
    CPW = 2 * UR + 2 * U * MW
    cphd = nc.declare_dram_parameter("cph", [56, CPW + 2 * N], f8, isOutput=False)
    # exact int32 tag template (HW iota rounds 10-bit values to a float grid)
    tagd = nc.declare_dram_parameter("tag", [P, 2 * N], i32, isOutput=False)
    idx_out = nc.declare_dram_parameter("idx", [BPC, UR, U * 16], i32, isOutput=True)

    # unit s = 7*b + u; groups: two solos (fill: each needs only a tag
    # half-tile), 12 pairs, then two solos so the tail drains finely
    groups = (
        [[0], [1]]
        + [[s, s + 1] for s in range(2, NS - 2, 2)]
        + [[NS - 2], [NS - 1]]
    )

    with TileContext(nc) as tc, ExitStack() as ctx:
        consts = ctx.enter_context(tc.tile_pool(name="consts", bufs=1))
        xt_pool = ctx.enter_context(tc.tile_pool(name="xt", bufs=2))
        fold_pool = ctx.enter_context(tc.tile_pool(name="fold", bufs=3))
        ps_mm = ctx.enter_context(tc.tile_pool(name="ps_mm", bufs=2, space="PSUM"))

        keyt = [consts.tile([P, 2 * N], i32, name=f"key_{i}") for i in range(NKEY)]
        cph = consts.tile([56, CPW + 2 * N], f8, name="cph")
        idx_acc = [
            consts.tile([P, U * 16], i32, name=f"idx_acc_{b}") for b in range(BPC)
        ]

        phiv = cph[:28, CPW:].rearrange("p (two n) -> p two n", two=2)
        ident = cph[:, 0 : 2 * UR].rearrange("p (two n) -> p two n", two=2)
        mskt = cph[:, 2 * UR : CPW].rearrange("p (two n) -> p two n", two=2)

        def prep(b, split):
            xt = xt_pool.tile([P, 4 * N], f8, tag="xt", name=f"x8_{b}")
            if split:
                # ktile halves: gram pair 0 can start after the first half
                nc.sync.dma_start(out=xt[:, 0 : 2 * N], in_=x8d.ap()[b, :, 0 : 2 * N])
                nc.sync.dma_start(out=xt[:, 2 * N :], in_=x8d.ap()[b, :, 2 * N :])
            else:
                nc.sync.dma_start(out=xt, in_=x8d.ap()[b])
            return xt.rearrange("p (k n) -> p k n", k=4)

        xks = {}

        def member_matmuls(ps, mi, b, u):
            xk = xks[b]
            r0 = u * UR
            co = mi * 1024
            j0 = _mask_j0(u)
            for c0, cw in CBLKS:
                nc.tensor.matmul(
                    ps[:UR, co + c0 : co + c0 + cw],
                    lhsT=xk[:, 0:2, r0 : r0 + UR],
                    rhs=xk[:, 0:2, c0 : c0 + cw],
                    start=True,
                    stop=False,
                    perf_mode=DR,
                )
                nc.tensor.matmul(
                    ps[:UR, co + c0 : co + c0 + cw],
                    lhsT=xk[:, 2:4, r0 : r0 + UR],
                    rhs=xk[:, 2:4, c0 : c0 + cw],
                    start=False,
                    stop=False,
                    perf_mode=DR,
                )
                a = max(j0, c0)
                z = min(j0 + MW, c0 + cw)
                if a < z:
                    nc.tensor.matmul(
                        ps[:UR, co + a : co + z],
                        lhsT=ident,
                        rhs=mskt[:, :, u * MW + (a - j0) : u * MW + (z - j0)],
                        start=False,
                        stop=False,
                        perf_mode=DR,
                    )
                nc.tensor.matmul(
                    ps[:UR, co + c0 : co + c0 + cw],
                    lhsT=phiv[:, :, r0 : r0 + UR],
                    rhs=phiv[:, :, c0 : c0 + cw],
                    start=False,
                    stop=True,
                    perf_mode=DR,
                )

        def emit_convert(gi, mem, ps):
            key = keyt[gi % NKEY]
            if len(mem) == 2:
                k16 = key.bitcast(i16).rearrange(
                    "p (two n half) -> p two n half", two=2, half=2
                )
                psv = ps.rearrange("p (two n) -> p two n", two=2)
                nc.scalar.activation(
                    k16[:UR, :, 0:ACOLS, 1], psv[:UR, :, 0:ACOLS], AF.Copy
                )
                if ACOLS < N:
                    with tc.high_priority():
                        nc.vector.tensor_copy(
                            k16[:UR, :, ACOLS:N, 1], psv[:UR, :, ACOLS:N]
                        )
            else:
                k16s = key.bitcast(i16).rearrange("p (n half) -> p n half", half=2)
                nc.scalar.activation(
                    k16s[:UR, 0:ACOLS, 1], ps[:UR, 0:ACOLS], AF.Copy
                )
                if ACOLS < N:
                    with tc.high_priority():
                        nc.vector.tensor_copy(
                            k16s[:UR, ACOLS:N, 1], ps[:UR, ACOLS:N]
                        )
            return key

        def emit_folds(gi, mem, key):
            # No fold stage: max8 runs directly on 392-wide mod-4 interleaved
            # column subsets ({0,1} and {2,3} mod 4 - spatially
            # anti-correlated, so a row's clustered neighbors split across
            # subsets). 16 exact per-subset candidates; host merges, keeps 10.
            kq = key.rearrange("p (two q four) -> p two q four", two=2, four=4)
            for mi, s in enumerate(mem):
                b, u = (None, None) if s == "T" else divmod(s, U)
                for half in range(2):
                    dst = (
                        idx_acc[b][:UR, u * 16 + 8 * half : u * 16 + 8 * half + 8]
                        if s != "T"
                        else idx_tail[:, 8 * half : 8 * half + 8]
                    )
                    nc.vector.max(
                        out=dst,
                        in_=kq[:UR, mi, :, 2 * half : 2 * half + 2],
                    )
                if s == "T":
                    continue
                # stream finished key columns out; the last batch goes in
                # two pieces so the post-last-unit tail is one small DMA
                cuts = (
                    {4: (0, 80), 6: (80, 112)}
                    if b == BPC - 1
                    else {5: (0, 96), 6: (96, 112)}
                )
                if u in cuts:
                    lo, hi = cuts[u]
                    nc.sync.dma_start(
                        out=idx_out.ap()[b, :, lo:hi], in_=idx_acc[b][:UR, lo:hi]
                    )

        # ---- driver ----
        # warm the ACT function table off the critical path
        warm = consts.tile([1, 2], f32, name="warm")
        nc.vector.memset(warm, 0.0)
        nc.scalar.activation(warm, warm, AF.Copy)
        xks[0] = prep(0, split=True)
        nc.sync.dma_start(out=cph, in_=cphd.ap())
        # exact tag templates by DMA (iota's float datapath rounds 10-bit
        # tags), in criticality order: the two leading solo groups touch only
        # the first halves of key0/key1; the pairs at groups 2/3 need the rest.
        nc.sync.dma_start(out=keyt[0][:, 0:N], in_=tagd.ap()[:, 0:N])
        nc.sync.dma_start(out=keyt[1][:, 0:N], in_=tagd.ap()[:, 0:N])
        nc.sync.dma_start(out=keyt[0][:, N:], in_=tagd.ap()[:, N:])
        nc.sync.dma_start(out=keyt[1][:, N:], in_=tagd.ap()[:, N:])

        for gi, mem in enumerate(groups):
            for s in mem:
                b, u = divmod(s, U)
                if u == 2 and b + 1 < BPC:
                    xks[b + 1] = prep(b + 1, split=False)
            ps = ps_mm.tile([P, 2048], f32, tag="ps_mm")
            for mi, s in enumerate(mem):
                member_matmuls(ps, mi, *divmod(s, U))
            key = emit_convert(gi, mem, ps)
            emit_folds(gi, mem, key)

    nc.finalize()
    return nc


def _get_nc():
    if "nc" not in _CACHE:
        _CACHE["nc"] = build_bass()
    return _CACHE["nc"]


def _host_consts():
    if "consts" in _CACHE:
        return _CACHE["consts"]
    mask = _mask_np()

    # c128 (DoubleRow layouts, contract d=(p,i)=2p+i): 96*I ++ mask bands
    identT = 96.0 * np.eye(UR, dtype=np.float32).reshape(56, 2, UR)
    mskDR = np.zeros((56, 2, U * MW), np.float32)
    for u in range(U):
        j0 = _mask_j0(u)
        mskDR[:, :, u * MW : (u + 1) * MW] = -224.0 * mask[
            u * UR : (u + 1) * UR, j0 : j0 + MW
        ].reshape(56, 2, MW)
    cph = np.zeros((56, 2 * UR + 2 * U * MW + 2 * N), np.float32)
    cph[:, 0 : 2 * UR] = identT.reshape(56, 2 * UR)
    cph[:, 2 * UR : 2 * UR + 2 * U * MW] = mskDR.reshape(56, 2 * U * MW)
    cph[:28, 2 * UR + 2 * U * MW :] = _phi_np().T.reshape(28, 2 * N)
    cph = cph.astype(F8NP)

    # tags spaced 64 apart: engine float datapaths round int32 keys to f32
    # (ulp <= 32 below 2^29), so a 64 spacing keeps tags exactly recoverable
    tag = np.broadcast_to(
        np.tile(64 * (1023 - np.arange(N, dtype=np.int32)), 2)[None, :], (P, 2 * N)
    ).copy()
    _CACHE["consts"] = (cph, tag)
    return _CACHE["consts"]


def kernel(node_feature, relative_pos):
    from concourse.bass_utils import run_bass_kernel_spmd

    x = np.asarray(node_feature, dtype=np.float32)

    nrm = np.sqrt((x * x).sum(-1, dtype=np.float32), dtype=np.float32)
    nrm = np.maximum(nrm, np.float32(1e-12))
    xh8 = (x * (np.float32(FSCALE) / nrm)[..., None]).astype(F8NP)  # [B, N, D]

    # ktile layout [B, 128, 4*784]: x8[b, p, i*784+n] = xh8[b, n, i*128+p]
    x8 = np.ascontiguousarray(
        xh8.reshape(BATCH, N, 4, P).transpose(0, 3, 2, 1).reshape(BATCH, P, 4 * N)
    )

    cph, tag = _host_consts()

    nc = _get_nc()
    in_maps = [
        {
            "x8": np.ascontiguousarray(x8[i * BPC : (i + 1) * BPC]),
            "cph": cph,
            "tag": tag,
        }
        for i in range(NCORES)
    ]
    res = run_bass_kernel_spmd(nc, in_maps, list(range(NCORES)))

    topk = np.zeros((BATCH, N, K), np.int32)
    for i in range(NCORES):
        keys = res.results[i]["idx"].reshape(BPC, UR, U, 16)
        # per unit: top8(h half0) ++ top8(h half1) -> merge, keep 10
        srt = np.sort(keys, axis=-1)[:, :, :, ::-1][:, :, :, :K]
        m = 1023 - (((srt & 65535) + 32) >> 6)
        topk[i * BPC : (i + 1) * BPC] = (
            m.transpose(0, 2, 1, 3).reshape(BPC, N, K).astype(np.int32)
        )

    dst = topk + (np.arange(BATCH, dtype=np.int32) * N)[:, None, None]
    src = np.broadcast_to(
        np.arange(BATCH * N, dtype=np.int32).reshape(BATCH, N, 1), (BATCH, N, K)
    )
    relation = np.zeros_like(dst)
    return np.stack([dst, src, relation], axis=-1).reshape(-1, 3)
